# revision 33
# baseline (speedup 1.0000x reference)
"""Trainium2 Bass kernel for EuclideanSimilarity:
out[i, j] = -||z_anc[i] - z_pos_neg[j]||_2
          = -sqrt(a2[i] + b2[j] - 2 * z_anc[i] . z_pos_neg[j])

Sharding: z_anc rows split across 8 cores (1024 rows each); z_pos_neg
replicated.  Each core computes a [1024, 8192] slab of the output.

Final design (~82.7us vs 85.8us bf16 baseline).  The wall is the ACT
(scalar) engine: every output element needs one table-sqrt pass, and at
~0.78ns/col + 261ns/instr the 32 [128,2048] tiles cost ~59us minimum.
The kernel runs that floor stall-free (1849ns/tile cadence):
  - PE: fp8e4 DoubleRow matmuls.  One K=256 contraction per 512-col
    psum bank fuses the ab GEMM and the b2 reduction: k-tile 0 =
    fp8(-2*SIG*aT) x fp8(bT), k-tile 1 = const 8.0 x sq',
    sq' = fp8((g*b)^2), g = sqrt(SIG/8), computed on-device by DVE
    squaring of the host-shipped fp8 g*bT.  psum = SIG*(b2-2ab); four
    DR matmuls per tile stream at ~215ns each, so the PE refills a
    psum buffer in ~1.1us and never paces the pipeline (2 psum bufs).
  - ACT: out = sqrt(psum/SIG + a2) per tile, bias = per-partition a2
    column (from one bf16 square + one segmented 3D tensor_reduce on
    DVE).  First and last tiles run as 2x1024 halves so the first
    output DMA issues ~1us earlier and the tail drains sooner.
  - DVE: fp16 negates (2x mode, ~0.69us/tile), emission deferred one
    tile so they never sit between psum consumers, plus the per-group
    sq' squares.  SIG=6.7877 kept from an earlier variant (any scale
    works for this path; it also matches the optional NEGSQRT custom
    DVE op registered below, which computes -sqrt(psum) in one 8-stage
    pass; measured net-negative here because a 2-deep PSUM cannot hide
    the matmul refill behind the longer DVE consumer, so DVE_TILES is
    empty).
  - DMA: inputs ~1.5MB fp8/bf16 split so the first-needed slices land
    first; all issues on Sync/Scalar queues (GPSIMD dma_start works but
    adds a multi-us DGE drain at teardown; GPSIMD tensor ops are ~40x
    slower than DVE and poison concurrent DVE ops -- never use them).
  - PE warmup matmuls bridge until real work arrives so the HAM
    throttle (1.67x clock) lifts before the first real matmuls.
  - End-to-end rel err (fro) 3.3e-3 vs the 2e-2 gate (fp8 quantization
    of the GEMM dominates; all reductions/bias are bf16/fp32).
"""

import os
import sys

import numpy as np

try:
    import concourse  # noqa: F401
except ImportError:
    for _p in ("/opt/trn_rl_repo", os.path.expanduser("~/.axon_site/_ro/trn_rl_repo")):
        if os.path.isdir(_p) and _p not in sys.path:
            sys.path.insert(0, _p)

import concourse.bass as bass  # noqa: F401
import concourse.mybir as mybir
import concourse.tile as tile
from concourse import bacc
from concourse import bass_utils
from concourse import dve_ops as _dv
from concourse.dve_spec import (
    C0,
    C1,
    C2,
    One,
    Spec,
    Src0,
    Src1,
    _has_src1,
    lower,
)
from concourse.dve_uop import DveOpSpec

N_CORES = 8
N, M, D = 8192, 8192, 128
R = N // N_CORES  # 1024 rows of z_anc per core
P = 128           # partitions
BANK = 512        # fp32 columns per PSUM bank
GRP = 2048        # columns per ACT/DVE/DMA group (4 banks)
MT = R // P       # 8 m-tiles per core
NG = M // GRP     # 4 n-groups

OUT_DT = mybir.dt.float16
_E4 = mybir.dt.np(mybir.dt.float8e4)
_BF16 = mybir.dt.np(mybir.dt.bfloat16)

# -sqrt fit constants: psum = SIG*d2; out = v*(1 - v*z0), v = psum*z0,
# z0 = (psum*CC2 + CC1)*psum + CC0 ~= -(1/sqrt3)*rsqrt(psum)
SIG = 6.78773589
CC2 = -2.00258100e-09  # x^2 coeff (imm2)
CC1 = 1.26261938e-05   # x coeff   (s0)
CC0 = -2.99214786e-02  # const     (s1)

# m-tiles per group on the DVE sqrt path; group 0 stays light so the
# DVE can run the a2/sq' startup chain while ACT drains early tiles
DVE_TILES = (frozenset(), frozenset(), frozenset(), frozenset())
W_WU = 11                          # PE warmup matmuls

_nc_cache = None
_ops_cache = None


def _register_dve_ops():
    """Register the custom DVE ops in concourse's runtime op registry.
    Idempotent; computes the pinned uops sha at registration time."""
    global _ops_cache
    if _ops_cache is not None:
        return _ops_cache

    def make(name, spec):
        if name in _dv._SUB_OPCODE_FOR_NAME:
            return next(o for o in _dv.OPS if o.name == name)
        row = _dv._CUSTOM_DVE_ROW_BASE + len(_dv.OPS)
        assert row < 0x20, "custom DVE opcode rows exhausted"
        shas = {}
        for ver in ("v3", "v4"):
            try:
                uops = lower(spec, ver=ver)
                shas[ver] = DveOpSpec(
                    name=name, opcode=row, uops=uops, rd1_en=_has_src1(spec)
                ).sha(ver)
            except Exception:
                if ver == "v3":
                    raise
        op = _dv.DveOp(name, spec, subdim=False, uops_sha=shas)
        _dv.OPS.append(op)
        _dv.CUSTOM_DVE_SPECS[name] = spec
        _dv._SUB_OPCODE_FOR_NAME[name] = row
        return op

    def _negsqrt_ref(in0, in1, s0, s1, imm2):
        x = in0.astype(np.float32)
        z0 = (x * np.float32(imm2) + np.float32(s0)) * x + np.float32(s1)
        v = x * z0
        return (v * (np.float32(1.0) - v * z0)).astype(np.float32)

    _z0 = (Src0 * C2 + C0) * Src0 + C1
    _v = Src0 * _z0
    negsqrt = make(
        "NEGSQRT_NR_ANT",
        Spec(body=_v * (One - _v * _z0), reference=_negsqrt_ref),
    )

    def _resid_ref(in0, in1, s0, s1, imm2):
        x = in0.astype(np.float32)
        q = in1.astype(np.float32)
        return ((x * np.float32(s0) - q * np.float32(s1))
                * np.float32(imm2)).astype(np.float32)

    resid = make(
        "RESID_SCALE_ANT",
        Spec(body=(Src0 * C0 - Src1 * C1) * C2, reference=_resid_ref),
    )
    _ops_cache = (negsqrt, resid)
    return _ops_cache


def _build():
    f32 = mybir.dt.float32
    bf16 = mybir.dt.bfloat16
    fp8 = mybir.dt.float8e4
    DR = mybir.MatmulPerfMode.DoubleRow
    negsqrt, resid = _register_dve_ops()

    nc = bacc.Bacc("TRN2", debug=False, target_bir_lowering=False)
    aw = nc.dram_tensor("aw", [P, 2, R], fp8, kind="ExternalInput").ap()
    aN = nc.dram_tensor("aN", [R, P], bf16, kind="ExternalInput").ap()
    bT = nc.dram_tensor("bT", [P, M], fp8, kind="ExternalInput").ap()
    bg = nc.dram_tensor("bg", [P, M], fp8, kind="ExternalInput").ap()
    out = nc.dram_tensor("out", [R, M], OUT_DT, kind="ExternalOutput").ap()

    with tile.TileContext(nc) as tc:
        with tc.tile_pool(name="consts", bufs=1) as consts:
            # spread the initial input DMAs over three issue queues, with
            # the first m-tile / first psum bank's slices first, so the
            # pipeline starts before the bulk of the inputs land
            bq = consts.tile([P, 2, M], fp8)   # ktile 0 = bT, ktile 1 = sq'
            bg_sb = consts.tile([P, M], fp8)
            # lhsT: ktile 0 = -2*SIG*aT, ktile 1 = 8.0
            aw_sb = consts.tile([P, 2, R], fp8)
            aN8 = consts.tile([P, R], bf16)  # [p, (t d)] for the a2 bias
            aN_r = bass.AP(
                tensor=aN.tensor, offset=aN.offset,
                ap=[[D, P], [P * D, MT], [1, D]],
            )
            nc.scalar.dma_start(
                out=aN8.rearrange("p (t d) -> p t d", d=D), in_=aN_r
            )
            nc.sync.dma_start(out=bg_sb[:, 0:GRP//2], in_=bg[:, 0:GRP//2])
            nc.scalar.dma_start(out=bq[:, 0, 0:GRP], in_=bT[:, 0:GRP])
            nc.sync.dma_start(out=aw_sb, in_=aw)
            nc.sync.dma_start(out=bg_sb[:, GRP//2:GRP], in_=bg[:, GRP//2:GRP])

            scratch = consts.tile([P, BANK], bf16)  # PE warmup fodder
            nc.gpsimd.memset(scratch, 0.001)
            junk = consts.tile([P, 8], f32)
            biasj = consts.tile([P, 1], f32)
            nc.gpsimd.memset(biasj, 1.0)
            asq = consts.tile([P, R], bf16)
            a2c = consts.tile([P, MT], f32)    # ACT bias columns (= a2)

            with (
                tc.tile_pool(name="mm", bufs=2, space="PSUM") as mm_pool,
                tc.tile_pool(name="o", bufs=4) as o_pool,
                tc.tile_pool(name="on", bufs=6) as on_pool,
            ):
                # preload the sqrt ACT table while DMAs are in flight
                nc.scalar.activation(
                    junk, scratch[:, 0:8], mybir.ActivationFunctionType.Sqrt,
                    bias=biasj[:, 0:1],
                )
                # PE warmup: HAM un-throttles after ~3.5us of activity
                wu = mm_pool.tile([P, GRP], f32, tag="ps")
                for k in range(W_WU):
                    nc.tensor.matmul(
                        wu[:, (k % 4) * BANK:(k % 4) * BANK + BANK],
                        lhsT=scratch[:, 0:P], rhs=scratch,
                        start=True, stop=True,
                    )

                # ---- a2 on DVE: one square + one segmented reduce --------
                nc.vector.tensor_mul(
                    bq[:, 1, 0:GRP // 2], bg_sb[:, 0:GRP // 2],
                    bg_sb[:, 0:GRP // 2]
                )
                nc.vector.tensor_mul(asq, aN8, aN8)
                nc.vector.tensor_reduce(
                    a2c.rearrange("p (t one) -> p t one", one=1),
                    asq.rearrange("p (t d) -> p t d", d=P),
                    axis=mybir.AxisListType.X, op=mybir.AluOpType.add,
                )
                nc.vector.tensor_mul(
                    bq[:, 1, GRP // 2:GRP], bg_sb[:, GRP // 2:GRP],
                    bg_sb[:, GRP // 2:GRP]
                )

                # ---- main loop (n-group-major) ---------------------------
                # negates only gate the out-DMA of an already-finished tile,
                # so defer their DVE emission until after the NEXT tile's
                # psum consumer — NEGSQRT then frees its psum banks ASAP
                # instead of queueing behind a negate
                pending = []

                def flush_pending():
                    while pending:
                        o_, on_, t_, g_ = pending.pop(0)
                        nc.vector.tensor_scalar_mul(on_, o_, -1.0)
                        nc.sync.dma_start(
                            out=out[t_ * P:(t_ + 1) * P,
                                    g_ * GRP:(g_ + 1) * GRP],
                            in_=on_,
                        )

                for g in range(NG):
                    if g + 1 < NG:
                        sl = slice((g + 1) * GRP, (g + 2) * GRP)
                        nc.sync.dma_start(out=bq[:, 0, sl], in_=bT[:, sl])
                        nc.sync.dma_start(out=bg_sb[:, sl], in_=bg[:, sl])
                    for t in range(MT):
                        ps = mm_pool.tile([P, GRP], f32, tag="ps")
                        for j in range(GRP // BANK):
                            c0 = g * GRP + j * BANK
                            nc.tensor.matmul(
                                ps[:, j * BANK:(j + 1) * BANK],
                                lhsT=aw_sb[:, :, t * P:(t + 1) * P],
                                rhs=bq[:, :, c0:c0 + BANK],
                                start=True, stop=True,
                                perf_mode=DR,
                            )
                        if t in DVE_TILES[g]:
                            on = on_pool.tile([P, GRP], OUT_DT, tag="on")
                            # two halves: the psum banks free progressively
                            # so the next tile's matmuls start sooner
                            for h in (slice(0, GRP // 2),
                                      slice(GRP // 2, GRP)):
                                nc.vector._custom_dve(
                                    negsqrt, out=on[:, h], in0=ps[:, h],
                                    s0=CC1, s1=CC0, imm2=CC2,
                                )
                            flush_pending()
                            nc.sync.dma_start(
                                out=out[t * P:(t + 1) * P,
                                        g * GRP:(g + 1) * GRP],
                                in_=on,
                            )
                        elif (g == 0 and t == 0) or (g == NG - 1 and t == MT - 1):
                            # first tile in halves: its output DMA starts
                            # before banks 2-3 of the psum are even filled
                            for h in (slice(0, GRP // 2),
                                      slice(GRP // 2, GRP)):
                                o = o_pool.tile([P, GRP], OUT_DT, tag="o")
                                nc.scalar.activation(
                                    o[:, h], ps[:, h],
                                    mybir.ActivationFunctionType.Sqrt,
                                    bias=a2c[:, t:t + 1], scale=1.0 / SIG,
                                )
                                on = on_pool.tile([P, GRP], OUT_DT,
                                                  tag="on")
                                nc.vector.tensor_scalar_mul(
                                    on[:, h], o[:, h], -1.0
                                )
                                nc.sync.dma_start(
                                    out=out[t * P:(t + 1) * P,
                                            g * GRP + h.start:
                                            g * GRP + h.stop],
                                    in_=on[:, h],
                                )
                        else:
                            o = o_pool.tile([P, GRP], OUT_DT, tag="o")
                            nc.scalar.activation(
                                o, ps, mybir.ActivationFunctionType.Sqrt,
                                bias=a2c[:, t:t + 1], scale=1.0 / SIG,
                            )
                            on = on_pool.tile([P, GRP], OUT_DT, tag="on")
                            pending.append((o, on, t, g))
                            if len(pending) > 1:
                                flush_pending()
                        # sq' for the NEXT group once its bg chunk landed
                        if t == 3 and g + 1 < NG:
                            flush_pending()
                            sl = slice((g + 1) * GRP, (g + 2) * GRP)
                            nc.vector.tensor_mul(
                                bq[:, 1, sl], bg_sb[:, sl], bg_sb[:, sl]
                            )
                flush_pending()

    nc.compile()
    return nc


def _get_nc():
    global _nc_cache
    if _nc_cache is None:
        _nc_cache = _build()
    return _nc_cache


def _in_maps(z_anc, z_pos_neg):
    za = np.asarray(z_anc, dtype=np.float32)
    zaT = np.ascontiguousarray(za.T)
    zbT = np.ascontiguousarray(np.asarray(z_pos_neg, dtype=np.float32).T)
    bT = zbT.astype(_E4)
    # bg rows squared feed the SIG*b2 fold: sq' = bg^2, W = 8.0
    gam = np.float32(np.sqrt(SIG / 8.0))
    bg = (zbT * gam).astype(_E4)
    awT = (zaT * np.float32(-2.0 * SIG)).astype(_E4)
    maps = []
    for c in range(N_CORES):
        rows = slice(c * R, (c + 1) * R)
        aw = np.empty((P, 2, R), dtype=_E4)
        aw[:, 0, :] = awT[:, rows]
        aw[:, 1, :] = np.float32(8.0)
        aNc = np.ascontiguousarray(za[rows, :]).astype(_BF16)
        maps.append({"aw": aw, "aN": aNc, "bT": bT, "bg": bg})
    return maps


def run(z_anc, z_pos_neg, **kwargs):
    """Run on hardware; returns (full_output, BassKernelResults)."""
    nc = _get_nc()
    res = bass_utils.run_bass_kernel_spmd(
        nc, _in_maps(z_anc, z_pos_neg), core_ids=list(range(N_CORES)), **kwargs
    )
    out = np.concatenate([r["out"] for r in res.results], axis=0)
    return out.astype(np.float32), res


def kernel(z_anc, z_pos_neg):
    out, _ = run(z_anc, z_pos_neg)
    return out
